# revision 1
# baseline (speedup 1.0000x reference)
"""Trainium2 Bass kernel for the chain-DAG generator MLP.

Math (per batch row b, node i in topological order 0..15):
    c_i = input_c @ Wc[:, 16i:16i+16]
    d_i = input_d @ theta[:, 16i:16i+16],  theta = mu + softplus(sigma)*noise_d
    h_i = relu(c_i @ W1c_i + d_i @ W1d_i + n_i @ W1n_i + p_i * w_p_i + b1_i)
    out_i = h_i @ W2_i + b2_i,   p_i = out_{i-1} for i in 1..13 (0,14,15 roots)

Device mapping (data-parallel over batch on 8 cores, B_s=16384 rows/core,
32 chunks of 512 batch columns, hidden-on-partition layout):
  - base per node-pair q: ONE K=48 fp32r matmul over [input_c^T; input_d^T;
    ones; pad; noise_2q^T; noise_2q+1^T] with folded weights (Wc/theta
    absorbed into the lhsT, b1 via the ones row) into PSUM bank [128, 512].
  - chain: relu(h_i) into a per-node SBUF tile at partitions 0:64;
    child_pre += outer(W2_i, w_p_{i+1})^T @ h_i — one K=64 matmul into the
    child's bank (M padded to 128; the parent scalar never materializes;
    b2_i folded into the child's bias row).
  - collect: per-node K=64 matmul (K=128 for the 12/13 and 14/15 tiles)
    accumulating W2-contractions into an output bank; rows 0:16 + b2 move
    to SBUF; DMA out transposed [16, B_s]; host transposes back.

HW constraint honored throughout: every matmul in one PSUM accumulation
group uses the same tile config — size (64,128) at position (0,0) — since
mixed tile positions inside a group corrupt execution, and fp32r cannot
column-tile (so M is always padded to 128 with zero weight columns).
"""

import threading

import numpy as np

import concourse.bacc as bacc
import concourse.mybir as mybir
from concourse.bass_utils import run_bass_kernel_spmd
from concourse.tile import TileContext

N_CORES = 8
B_FULL = 131072
B_S = B_FULL // N_CORES  # 16384
CHUNK = 512
I_DIM = 16
N_PAIRS = 8

F32 = mybir.dt.float32
FR = mybir.dt.float32r


def build_nc(b_s: int = B_S, chunk: int = CHUNK):
    """Build the single-core program (SPMD: same program on all cores)."""
    assert b_s % chunk == 0
    n_chunks = b_s // chunk

    nc = bacc.Bacc(
        "TRN2", target_bir_lowering=False, debug=False, num_devices=N_CORES
    )

    # Per-core inputs
    s_d = nc.dram_tensor("S", [16, b_s], FR, kind="ExternalInput").ap()
    nt_d = nc.dram_tensor("NT", [256, b_s], FR, kind="ExternalInput").ap()
    # Folded weights (replicated on every core)
    px_d = nc.dram_tensor("PX", [48, 128 * N_PAIRS], FR, kind="ExternalInput").ap()
    mc_d = nc.dram_tensor("MC", [64, 128 * 13], FR, kind="ExternalInput").ap()
    cl_d = nc.dram_tensor("CLW", [64, 128 * 16], FR, kind="ExternalInput").ap()
    b2_d = nc.dram_tensor("B2", [16, 1], F32, kind="ExternalInput").ap()
    out_d = nc.dram_tensor("OUT", [16, b_s], F32, kind="ExternalOutput").ap()

    with TileContext(nc) as tc:
        with (
            tc.tile_pool(name="consts", bufs=1) as cpool,
            tc.tile_pool(name="ins", bufs=20) as ipool,
            tc.tile_pool(name="hbuf", bufs=18) as hpool,
            tc.tile_pool(name="obuf", bufs=3) as opool,
            tc.tile_pool(name="pairs", bufs=6, space="PSUM") as ppool,
            tc.tile_pool(name="outp", bufs=2, space="PSUM") as qpool,
        ):
            px_t = cpool.tile([48, 128 * N_PAIRS], FR)
            nc.sync.dma_start(out=px_t[:, :], in_=px_d[:, :])
            mc_t = cpool.tile([64, 128 * 13], FR)
            nc.sync.dma_start(out=mc_t[:, :], in_=mc_d[:, :])
            cl_t = cpool.tile([64, 128 * 16], FR)
            nc.sync.dma_start(out=cl_t[:, :], in_=cl_d[:, :])
            b2_t = cpool.tile([16, 1], F32)
            nc.sync.dma_start(out=b2_t[:, :], in_=b2_d[:, :])

            for ch in range(n_chunks):
                c0 = ch * chunk
                sl = slice(c0, c0 + chunk)

                # --- base: one K=48 matmul per pair bank ---
                banks = []
                for q in range(N_PAIRS):
                    x_q = ipool.tile([48, chunk], FR, tag="x", name=f"x_{ch}_{q}")
                    nc.sync.dma_start(out=x_q[0:16, :], in_=s_d[:, sl])
                    nc.sync.dma_start(
                        out=x_q[16:48, :], in_=nt_d[32 * q : 32 * q + 32, sl]
                    )
                    bank = ppool.tile(
                        [128, chunk], F32, tag="bank", name=f"bank_{ch}_{q}"
                    )
                    banks.append(bank)
                    nc.tensor.matmul(
                        out=bank[:, :],
                        lhsT=px_t[:, 128 * q : 128 * (q + 1)],
                        rhs=x_q[:, :],
                        start=True,
                        stop=(q == 7),  # bank 7 takes no chain matmul
                        skip_group_check=True,
                    )

                # --- chain + collect ---
                # h tiles: one [64, chunk] per node, always at partitions
                # 0:64 so every chain/collect matmul runs at tile config
                # (64, 128) @ (0, 0).
                bank_out = qpool.tile([128, chunk], F32, tag="bout")
                for i in range(I_DIM):
                    q, r = divmod(i, 2)
                    h = hpool.tile([64, chunk], FR, tag="h", name=f"h_{ch}_{i}")
                    brows = slice(64 * r, 64 * (r + 1))
                    if i % 2 == 0:
                        nc.scalar.activation(
                            h[:, :],
                            banks[q][brows, :],
                            mybir.ActivationFunctionType.Relu,
                        )
                    else:
                        nc.vector.tensor_scalar_max(
                            out=h[:, :], in0=banks[q][brows, :], scalar1=0.0
                        )
                    if i <= 12:
                        rc = (i + 1) % 2
                        nc.tensor.matmul(
                            out=banks[(i + 1) // 2][:, :],
                            lhsT=mc_t[:, 128 * i : 128 * (i + 1)],
                            rhs=h[:, :],
                            start=False,
                            stop=(rc == 1),  # chain(2q) closes pair q's group
                            skip_group_check=True,
                        )
                    nc.tensor.matmul(
                        out=bank_out[:, :],
                        lhsT=cl_t[:, 128 * i : 128 * (i + 1)],
                        rhs=h[:, :],
                        start=(i == 0),
                        stop=(i == 15),
                        skip_group_check=True,
                    )

                o_t = opool.tile([16, chunk], F32, tag="o")
                nc.vector.tensor_scalar_add(
                    out=o_t[:, :], in0=bank_out[0:16, :], scalar1=b2_t[:, 0:1]
                )
                nc.sync.dma_start(out=out_d[:, sl], in_=o_t[:, :])

    nc.compile()
    return nc


def prep_weights(noise_d, mu, sigma, Wc, W1, b1, W2, b2):
    """Fold the tiny parameter tensors into the device weight layout."""
    theta = mu + np.log1p(np.exp(sigma)) * noise_d  # [4, 256]
    w_p = W1[:, 48, :]  # [16, 64]
    b1e = b1.copy()  # [16, 64]
    for i in range(1, 14):  # nodes with parent i-1
        b1e[i] = b1[i] + w_p[i] * b2[i - 1]

    # base lhsT per pair: rows [A_c(10); A_d(4); b1e(1); 0(1); A_n block-diag(32)]
    px = np.zeros((48, 128 * N_PAIRS), np.float32)
    for q in range(N_PAIRS):
        for r in range(2):
            i = 2 * q + r
            cols = slice(128 * q + 64 * r, 128 * q + 64 * (r + 1))
            px[0:10, cols] = Wc[:, 16 * i : 16 * (i + 1)] @ W1[i, 0:16, :]
            px[10:14, cols] = theta[:, 16 * i : 16 * (i + 1)] @ W1[i, 16:32, :]
            px[14, cols] = b1e[i]
            px[16 + 16 * r : 32 + 16 * r, cols] = W1[i, 32:48, :]

    # chain lhsT for node i -> child i+1 (child's rows at 64*((i+1)%2))
    mc = np.zeros((64, 128 * 13), np.float32)
    for i in range(13):
        c0 = 128 * i + 64 * ((i + 1) % 2)
        mc[:, c0 : c0 + 64] = np.outer(W2[i], w_p[i + 1])

    # collect lhsT: one [64, 128] block per node; real column = node id
    cl = np.zeros((64, 128 * 16), np.float32)
    for i in range(16):
        cl[:, 128 * i + i] = W2[i]

    return {
        "PX": px,
        "MC": mc,
        "CLW": cl,
        "B2": b2.reshape(16, 1).astype(np.float32),
    }


def prep_core_inputs(noise, input_c, input_d, c):
    """Shard + transpose per-core batch inputs."""
    b0, b1_ = c * B_S, (c + 1) * B_S
    s = np.zeros((16, B_S), np.float32)
    s[0:10] = input_c[b0:b1_].T
    s[10:14] = input_d[b0:b1_].T
    s[14] = 1.0
    nt = np.ascontiguousarray(noise[b0:b1_].T)
    return {"S": s, "NT": nt}


_NC_LOCK = threading.Lock()
_NC_CACHE = {}


def _get_nc():
    with _NC_LOCK:
        if "nc" not in _NC_CACHE:
            _NC_CACHE["nc"] = build_nc()
        return _NC_CACHE["nc"]


def kernel(noise, input_c, input_d, noise_d, mu, sigma, Wc, W1, b1, W2, b2):
    noise = np.asarray(noise, np.float32)
    input_c = np.asarray(input_c, np.float32)
    input_d = np.asarray(input_d, np.float32)
    w = prep_weights(
        np.asarray(noise_d, np.float32),
        np.asarray(mu, np.float32),
        np.asarray(sigma, np.float32),
        np.asarray(Wc, np.float32),
        np.asarray(W1, np.float32),
        np.asarray(b1, np.float32),
        np.asarray(W2, np.float32),
        np.asarray(b2, np.float32),
    )
    in_maps = []
    for c in range(N_CORES):
        m = prep_core_inputs(noise, input_c, input_d, c)
        m.update(w)
        in_maps.append(m)

    nc = _get_nc()
    res = run_bass_kernel_spmd(nc, in_maps, list(range(N_CORES)))
    out = np.concatenate(
        [res.results[c]["OUT"].T for c in range(N_CORES)], axis=0
    )
    return np.ascontiguousarray(out, np.float32)



# revision 4
# speedup vs baseline: 1.1111x; 1.1111x over previous
"""Trainium2 Bass kernel for the chain-DAG generator MLP.

Math (per batch row b, node i in topological order 0..15):
    c_i = input_c @ Wc[:, 16i:16i+16]
    d_i = input_d @ theta[:, 16i:16i+16],  theta = mu + softplus(sigma)*noise_d
    h_i = relu(c_i @ W1c_i + d_i @ W1d_i + n_i @ W1n_i + p_i * w_p_i + b1_i)
    out_i = h_i @ W2_i + b2_i,   p_i = out_{i-1} for i in 1..13 (0,14,15 roots)

Device mapping (data-parallel over batch on 8 cores, B_s=16384 rows/core,
32 chunks of 512 batch columns, hidden-on-partition layout):
  - base per node-pair q: ONE K=48 fp32r matmul over [input_c^T; input_d^T;
    ones; pad; noise_2q^T; noise_2q+1^T] with folded weights (Wc/theta
    absorbed into the lhsT, b1 via the ones row) into PSUM bank [128, 512].
  - chain: relu(h_i) into a per-node SBUF tile at partitions 0:64;
    child_pre += outer(W2_i, w_p_{i+1})^T @ h_i — one K=64 matmul into the
    child's bank (M padded to 128; the parent scalar never materializes;
    b2_i folded into the child's bias row).
  - collect: per-node K=64 matmul (K=128 for the 12/13 and 14/15 tiles)
    accumulating W2-contractions into an output bank; rows 0:16 + b2 move
    to SBUF; DMA out transposed [16, B_s]; host transposes back.

HW constraint honored throughout: every matmul in one PSUM accumulation
group uses the same tile config — size (64,128) at position (0,0) — since
mixed tile positions inside a group corrupt execution, and fp32r cannot
column-tile (so M is always padded to 128 with zero weight columns).
"""

import threading

import ml_dtypes
import numpy as np

import concourse.bacc as bacc
import concourse.mybir as mybir
from concourse.bass_utils import run_bass_kernel_spmd
from concourse.tile import TileContext

N_CORES = 8
B_FULL = 131072
B_S = B_FULL // N_CORES  # 16384
CHUNK = 512
I_DIM = 16
N_PAIRS = 8

F32 = mybir.dt.float32
FR = mybir.dt.bfloat16
BF16 = ml_dtypes.bfloat16


def build_nc(b_s: int = B_S, chunk: int = CHUNK):
    """Build the single-core program (SPMD: same program on all cores)."""
    assert b_s % chunk == 0
    n_chunks = b_s // chunk

    nc = bacc.Bacc(
        "TRN2", target_bir_lowering=False, debug=False, num_devices=N_CORES
    )

    # Per-core inputs
    s_d = nc.dram_tensor("S", [16, b_s], FR, kind="ExternalInput").ap()
    nt_d = nc.dram_tensor("NT", [256, b_s], FR, kind="ExternalInput").ap()
    # Folded weights (replicated on every core)
    px_d = nc.dram_tensor("PX", [48, 128 * N_PAIRS], FR, kind="ExternalInput").ap()
    mc_d = nc.dram_tensor("MC", [64, 128 * 13], FR, kind="ExternalInput").ap()
    cl_d = nc.dram_tensor("CLW", [64, 128 * 16], FR, kind="ExternalInput").ap()
    b2_d = nc.dram_tensor("B2", [16, 1], F32, kind="ExternalInput").ap()
    out_d = nc.dram_tensor("OUT", [16, b_s], F32, kind="ExternalOutput").ap()

    with TileContext(nc) as tc:
        with (
            tc.tile_pool(name="consts", bufs=1) as cpool,
            tc.tile_pool(name="ins", bufs=20) as ipool,
            tc.tile_pool(name="hbuf", bufs=18) as hpool,
            tc.tile_pool(name="obuf", bufs=3) as opool,
            tc.tile_pool(name="pairs", bufs=6, space="PSUM") as ppool,
            tc.tile_pool(name="outp", bufs=2, space="PSUM") as qpool,
        ):
            px_t = cpool.tile([48, 128 * N_PAIRS], FR)
            nc.sync.dma_start(out=px_t[:, :], in_=px_d[:, :])
            mc_t = cpool.tile([64, 128 * 13], FR)
            nc.sync.dma_start(out=mc_t[:, :], in_=mc_d[:, :])
            cl_t = cpool.tile([64, 128 * 16], FR)
            nc.sync.dma_start(out=cl_t[:, :], in_=cl_d[:, :])
            b2_t = cpool.tile([16, 1], F32)
            nc.sync.dma_start(out=b2_t[:, :], in_=b2_d[:, :])

            for ch in range(n_chunks):
                c0 = ch * chunk
                sl = slice(c0, c0 + chunk)

                # --- base: one K=48 matmul per pair bank ---
                banks = []
                for q in range(N_PAIRS):
                    x_q = ipool.tile([48, chunk], FR, tag="x", name=f"x_{ch}_{q}")
                    nc.sync.dma_start(out=x_q[0:16, :], in_=s_d[:, sl])
                    nc.sync.dma_start(
                        out=x_q[16:48, :], in_=nt_d[32 * q : 32 * q + 32, sl]
                    )
                    bank = ppool.tile(
                        [128, chunk], F32, tag="bank", name=f"bank_{ch}_{q}"
                    )
                    banks.append(bank)
                    nc.tensor.matmul(
                        out=bank[:, :],
                        lhsT=px_t[:, 128 * q : 128 * (q + 1)],
                        rhs=x_q[:, :],
                        start=True,
                        stop=(q == 7),  # bank 7 takes no chain matmul
                        skip_group_check=True,
                    )

                # --- chain + collect ---
                # h tiles: one [64, chunk] per node, always at partitions
                # 0:64 so every chain/collect matmul runs at tile config
                # (64, 128) @ (0, 0).
                bank_out = qpool.tile([128, chunk], F32, tag="bout")
                for i in range(I_DIM):
                    q, r = divmod(i, 2)
                    h = hpool.tile([64, chunk], FR, tag="h", name=f"h_{ch}_{i}")
                    brows = slice(64 * r, 64 * (r + 1))
                    if i % 2 == 0:
                        nc.scalar.activation(
                            h[:, :],
                            banks[q][brows, :],
                            mybir.ActivationFunctionType.Relu,
                        )
                    else:
                        nc.vector.tensor_scalar_max(
                            out=h[:, :], in0=banks[q][brows, :], scalar1=0.0
                        )
                    if i <= 12:
                        rc = (i + 1) % 2
                        nc.tensor.matmul(
                            out=banks[(i + 1) // 2][:, :],
                            lhsT=mc_t[:, 128 * i : 128 * (i + 1)],
                            rhs=h[:, :],
                            start=False,
                            stop=(rc == 1),  # chain(2q) closes pair q's group
                            skip_group_check=True,
                        )
                    nc.tensor.matmul(
                        out=bank_out[:, :],
                        lhsT=cl_t[:, 128 * i : 128 * (i + 1)],
                        rhs=h[:, :],
                        start=(i == 0),
                        stop=(i == 15),
                        skip_group_check=True,
                    )

                o_t = opool.tile([16, chunk], F32, tag="o")
                nc.vector.tensor_scalar_add(
                    out=o_t[:, :], in0=bank_out[0:16, :], scalar1=b2_t[:, 0:1]
                )
                nc.sync.dma_start(out=out_d[:, sl], in_=o_t[:, :])

    nc.compile()
    return nc


def prep_weights(noise_d, mu, sigma, Wc, W1, b1, W2, b2):
    """Fold the tiny parameter tensors into the device weight layout."""
    theta = mu + np.log1p(np.exp(sigma)) * noise_d  # [4, 256]
    w_p = W1[:, 48, :]  # [16, 64]
    b1e = b1.copy()  # [16, 64]
    for i in range(1, 14):  # nodes with parent i-1
        b1e[i] = b1[i] + w_p[i] * b2[i - 1]

    # base lhsT per pair: rows [A_c(10); A_d(4); b1e(1); 0(1); A_n block-diag(32)]
    px = np.zeros((48, 128 * N_PAIRS), np.float32)
    for q in range(N_PAIRS):
        for r in range(2):
            i = 2 * q + r
            cols = slice(128 * q + 64 * r, 128 * q + 64 * (r + 1))
            px[0:10, cols] = Wc[:, 16 * i : 16 * (i + 1)] @ W1[i, 0:16, :]
            px[10:14, cols] = theta[:, 16 * i : 16 * (i + 1)] @ W1[i, 16:32, :]
            px[14, cols] = b1e[i]
            px[16 + 16 * r : 32 + 16 * r, cols] = W1[i, 32:48, :]

    # chain lhsT for node i -> child i+1 (child's rows at 64*((i+1)%2))
    mc = np.zeros((64, 128 * 13), np.float32)
    for i in range(13):
        c0 = 128 * i + 64 * ((i + 1) % 2)
        mc[:, c0 : c0 + 64] = np.outer(W2[i], w_p[i + 1])

    # collect lhsT: one [64, 128] block per node; real column = node id
    cl = np.zeros((64, 128 * 16), np.float32)
    for i in range(16):
        cl[:, 128 * i + i] = W2[i]

    return {
        "PX": px.astype(BF16),
        "MC": mc.astype(BF16),
        "CLW": cl.astype(BF16),
        "B2": b2.reshape(16, 1).astype(np.float32),
    }


def prep_core_inputs(noise, input_c, input_d, c):
    """Shard + transpose per-core batch inputs."""
    b0, b1_ = c * B_S, (c + 1) * B_S
    s = np.zeros((16, B_S), np.float32)
    s[0:10] = input_c[b0:b1_].T
    s[10:14] = input_d[b0:b1_].T
    s[14] = 1.0
    nt = np.ascontiguousarray(noise[b0:b1_].T)
    return {"S": s.astype(BF16), "NT": nt.astype(BF16)}


_NC_LOCK = threading.Lock()
_NC_CACHE = {}


def _get_nc():
    with _NC_LOCK:
        if "nc" not in _NC_CACHE:
            _NC_CACHE["nc"] = build_nc()
        return _NC_CACHE["nc"]


def kernel(noise, input_c, input_d, noise_d, mu, sigma, Wc, W1, b1, W2, b2):
    noise = np.asarray(noise, np.float32)
    input_c = np.asarray(input_c, np.float32)
    input_d = np.asarray(input_d, np.float32)
    w = prep_weights(
        np.asarray(noise_d, np.float32),
        np.asarray(mu, np.float32),
        np.asarray(sigma, np.float32),
        np.asarray(Wc, np.float32),
        np.asarray(W1, np.float32),
        np.asarray(b1, np.float32),
        np.asarray(W2, np.float32),
        np.asarray(b2, np.float32),
    )
    in_maps = []
    for c in range(N_CORES):
        m = prep_core_inputs(noise, input_c, input_d, c)
        m.update(w)
        in_maps.append(m)

    nc = _get_nc()
    res = run_bass_kernel_spmd(nc, in_maps, list(range(N_CORES)))
    out = np.concatenate(
        [res.results[c]["OUT"].T for c in range(N_CORES)], axis=0
    )
    return np.ascontiguousarray(out, np.float32)



# revision 11
# speedup vs baseline: 1.2373x; 1.1136x over previous
"""Trainium2 Bass kernel for the chain-DAG generator MLP.

Math (per batch row b, node i in topological order 0..15):
    c_i = input_c @ Wc[:, 16i:16i+16]
    d_i = input_d @ theta[:, 16i:16i+16],  theta = mu + softplus(sigma)*noise_d
    h_i = relu(c_i @ W1c_i + d_i @ W1d_i + n_i @ W1n_i + p_i * w_p_i + b1_i)
    out_i = h_i @ W2_i + b2_i,   p_i = out_{i-1} for i in 1..13 (0,14,15 roots)

Device mapping (data-parallel over batch on 8 cores, B_s=16384 rows/core,
32 chunks of 512 batch columns, hidden-on-partition layout, all-bf16
operands with fp32 PSUM accumulation):

  - Two-strip PE row tiling: every matmul is tile size (64,128) at row
    position 64*(q%2) for target bank q — matmuls on opposite strips
    execute concurrently in the PE array (independent 32-row groups).
    Node i's hidden rows sit at bank rows 64*s_i, s_i = ((i+1)//2)%2, so
    the two nodes of a pair always occupy opposite halves and every rhs
    (x_q, h_i) lives at SBUF partitions 64*strip matching its target
    bank's group position.
  - base per pair q: ONE K=48 bf16 matmul (Wc/theta/b1 folded into lhsT,
    bias via ones row) into PSUM bank [128, 512].
  - chain: relu(h_i) (Act/DVE round-robin) into SBUF at partitions
    64*s_i; child_pre += outer(W2_i, w_p_{i+1})^T @ h_i; parent b2
    folded into child bias on host.
  - collect: at chunk end, 16 K=64 matmuls interleaved across strips
    into two out banks (A: s_i=0 nodes, B: s_i=1), merged by one DVE
    tensor_add into [16,512] fp32; b2 added on host after gather.
  - 2-chunk software pipelining: chunks are emitted staggered by 8 node
    steps so the PE instruction stream stays dense (HAM stays warm) and
    chain-relu latency of one chunk hides behind the other's matmuls.
"""

import threading

import ml_dtypes
import numpy as np

import concourse.bacc as bacc
import concourse.mybir as mybir
from concourse.bass_utils import run_bass_kernel_spmd
from concourse.tile import TileContext

N_CORES = 8
B_FULL = 131072
B_S = B_FULL // N_CORES  # 16384
CHUNK = 512
I_DIM = 16
N_PAIRS = 8
STAG = 8  # node-step stagger between in-flight chunks

F32 = mybir.dt.float32
BF = mybir.dt.bfloat16
BF16 = ml_dtypes.bfloat16

# row strip (0/1) of node i's hidden inside its pair bank; also the SBUF
# strip of h_i and the PE row group of every matmul reading h_i.
S_STRIP = [((i + 1) // 2) % 2 for i in range(I_DIM)]
# PE row group of all matmuls accumulating into pair bank q
P_STRIP = [q % 2 for q in range(N_PAIRS)]
COLLECT_A = [i for i in range(I_DIM) if S_STRIP[i] == 0]
COLLECT_B = [i for i in range(I_DIM) if S_STRIP[i] == 1]


def build_nc(b_s: int = B_S, chunk: int = CHUNK):
    """Build the single-core program (SPMD: same program on all cores)."""
    assert b_s % chunk == 0
    n_chunks = b_s // chunk

    nc = bacc.Bacc(
        "TRN2", target_bir_lowering=False, debug=False, num_devices=N_CORES
    )

    # Per-core inputs
    s_d = nc.dram_tensor("S", [16, b_s], BF, kind="ExternalInput").ap()
    nt_d = nc.dram_tensor("NT", [256, b_s], BF, kind="ExternalInput").ap()
    # Folded weights (replicated on every core)
    px_d = nc.dram_tensor("PX", [128, 128 * N_PAIRS], BF, kind="ExternalInput").ap()
    mc_d = nc.dram_tensor("MC", [128, 128 * 13], BF, kind="ExternalInput").ap()
    cl_d = nc.dram_tensor("CLW", [128, 128 * 16], BF, kind="ExternalInput").ap()
    out_d = nc.dram_tensor("OUT", [16, b_s], F32, kind="ExternalOutput").ap()

    with TileContext(nc) as tc:
        with (
            tc.tile_pool(name="consts", bufs=1) as cpool,
            tc.tile_pool(name="ins", bufs=18) as ipool,
            tc.tile_pool(name="hbuf", bufs=26) as hpool,
            tc.tile_pool(name="obuf", bufs=3) as opool,
            tc.tile_pool(name="pairs", bufs=6, space="PSUM") as ppool,
            tc.tile_pool(name="outp", bufs=1, space="PSUM") as qpool,
        ):
            px_t = cpool.tile([128, 128 * N_PAIRS], BF)
            nc.sync.dma_start(out=px_t[:, :], in_=px_d[:, :])
            mc_t = cpool.tile([128, 128 * 13], BF)
            nc.sync.dma_start(out=mc_t[:, :], in_=mc_d[:, :])
            cl_t = cpool.tile([128, 128 * 16], BF)
            nc.sync.dma_start(out=cl_t[:, :], in_=cl_d[:, :])

            # per-chunk state, keyed by chunk index
            banks = {}  # (ch, q) -> pair bank tile
            hs = {}  # (ch, i) -> h tile
            xs = {}  # (ch, q) -> x tile
            relu_rr = [0]  # global round-robin over Act/DVE

            def emit_base(ch, q):
                c0 = ch * chunk
                sl = slice(c0, c0 + chunk)
                R = 64 * P_STRIP[q]
                x_q = ipool.tile([128, chunk], BF, tag="x", name=f"x_{ch}_{q}")
                nc.sync.dma_start(out=x_q[R : R + 16, :], in_=s_d[:, sl])
                nc.sync.dma_start(
                    out=x_q[R + 16 : R + 48, :], in_=nt_d[32 * q : 32 * q + 32, sl]
                )
                xs[(ch, q)] = x_q
                bank = ppool.tile(
                    [128, chunk], F32, tag="bank", name=f"bank_{ch}_{q}"
                )
                banks[(ch, q)] = bank
                nc.tensor.matmul(
                    out=bank[:, :],
                    lhsT=px_t[R : R + 48, 128 * q : 128 * (q + 1)],
                    rhs=x_q[R : R + 48, :],
                    start=True,
                    stop=(q == 7),  # bank 7 takes no chain matmul
                    skip_group_check=True,
                )

            def emit_node(ch, i):
                q, s = i // 2, S_STRIP[i]
                if i == 0:
                    emit_base(ch, 0)
                if i % 2 == 1 and (i + 1) // 2 <= 7:
                    emit_base(ch, (i + 1) // 2)
                # relu: PSUM bank rows -> SBUF h tile, same partition range
                h = hpool.tile([128, chunk], BF, tag="h", name=f"h_{ch}_{i}")
                hs[(ch, i)] = h
                rows = slice(64 * s, 64 * s + 64)
                if relu_rr[0] % 2 == 0:
                    nc.scalar.activation(
                        h[rows, :],
                        banks[(ch, q)][rows, :],
                        mybir.ActivationFunctionType.Relu,
                    )
                else:
                    nc.vector.tensor_scalar_max(
                        out=h[rows, :], in0=banks[(ch, q)][rows, :], scalar1=0.0
                    )
                relu_rr[0] += 1
                if i <= 12:
                    sc = S_STRIP[i + 1]
                    nc.tensor.matmul(
                        out=banks[(ch, (i + 1) // 2)][:, :],
                        lhsT=mc_t[rows, 128 * i : 128 * (i + 1)],
                        rhs=h[rows, :],
                        start=False,
                        stop=(i % 2 == 0),  # chain(2q) closes pair q's group
                        skip_group_check=True,
                    )
                if i == 15:
                    emit_collect(ch)

            def emit_collect(ch):
                c0 = ch * chunk
                sl = slice(c0, c0 + chunk)
                bo = {
                    0: qpool.tile([128, chunk], F32, tag="boA", name=f"boA_{ch}"),
                    1: qpool.tile([128, chunk], F32, tag="boB", name=f"boB_{ch}"),
                }
                for k in range(8):
                    for g, grp in ((0, COLLECT_A), (1, COLLECT_B)):
                        i = grp[k]
                        rows = slice(64 * S_STRIP[i], 64 * S_STRIP[i] + 64)
                        nc.tensor.matmul(
                            out=bo[g][:, :],
                            lhsT=cl_t[rows, 128 * i : 128 * (i + 1)],
                            rhs=hs[(ch, i)][rows, :],
                            start=(k == 0),
                            stop=(k == 7),
                            skip_group_check=True,
                        )
                # each group lands on partitions 0:8 of its own bank
                # (row->node permutation undone on host)
                o_ta = opool.tile([8, chunk], F32, tag="oa")
                o_tb = opool.tile([8, chunk], F32, tag="ob")
                nc.scalar.copy(out=o_ta[:, :], in_=bo[0][0:8, :])
                nc.vector.tensor_copy(out=o_tb[:, :], in_=bo[1][0:8, :])
                nc.sync.dma_start(out=out_d[0:8, sl], in_=o_ta[:, :])
                nc.sync.dma_start(out=out_d[8:16, sl], in_=o_tb[:, :])
                # drop finished-chunk references
                for kk in [k for k in banks if k[0] == ch]:
                    del banks[kk]
                for kk in [k for k in hs if k[0] == ch]:
                    del hs[kk]
                for kk in [k for k in xs if k[0] == ch]:
                    del xs[kk]

            n_steps = I_DIM + STAG * (n_chunks - 1)
            for t in range(n_steps):
                for ch in range(n_chunks):
                    i = t - STAG * ch
                    if 0 <= i < I_DIM:
                        emit_node(ch, i)

    nc.compile()
    return nc


def prep_weights(noise_d, mu, sigma, Wc, W1, b1, W2, b2):
    """Fold the tiny parameter tensors into the device weight layout."""
    theta = mu + np.log1p(np.exp(sigma)) * noise_d  # [4, 256]
    w_p = W1[:, 48, :]  # [16, 64]
    b1e = b1.copy()  # [16, 64]
    for i in range(1, 14):  # nodes with parent i-1
        b1e[i] = b1[i] + w_p[i] * b2[i - 1]

    # base lhsT per pair at rows 64*P_STRIP[q]:
    # [A_c(10); A_d(4); b1e(1); 0(1); A_n block-diag(32)]; node i's 64
    # M-columns at 64*S_STRIP[i].
    px = np.zeros((128, 128 * N_PAIRS), np.float32)
    for q in range(N_PAIRS):
        R = 64 * P_STRIP[q]
        for r in range(2):
            i = 2 * q + r
            cols = slice(128 * q + 64 * S_STRIP[i], 128 * q + 64 * S_STRIP[i] + 64)
            px[R + 0 : R + 10, cols] = Wc[:, 16 * i : 16 * (i + 1)] @ W1[i, 0:16, :]
            px[R + 10 : R + 14, cols] = (
                theta[:, 16 * i : 16 * (i + 1)] @ W1[i, 16:32, :]
            )
            px[R + 14, cols] = b1e[i]
            px[R + 16 + 16 * r : R + 32 + 16 * r, cols] = W1[i, 32:48, :]

    # chain lhsT for node i -> child i+1 at rows 64*S_STRIP[i]; child's
    # M-columns at 64*S_STRIP[i+1]
    mc = np.zeros((128, 128 * 13), np.float32)
    for i in range(13):
        R = 64 * S_STRIP[i]
        c0 = 128 * i + 64 * S_STRIP[i + 1]
        mc[R : R + 64, c0 : c0 + 64] = np.outer(W2[i], w_p[i + 1])

    # collect lhsT: one [64, 128] block per node at rows 64*S_STRIP[i];
    # real column = position in PERM (A-nodes at 0:8, B-nodes at 8:16)
    cl = np.zeros((128, 128 * 16), np.float32)
    for i in range(16):
        R = 64 * S_STRIP[i]
        grp = COLLECT_A if S_STRIP[i] == 0 else COLLECT_B
        cl[R : R + 64, 128 * i + grp.index(i)] = W2[i]

    return {
        "PX": px.astype(BF16),
        "MC": mc.astype(BF16),
        "CLW": cl.astype(BF16),
    }


def prep_core_inputs(noise, input_c, input_d, c):
    """Shard + transpose per-core batch inputs."""
    b0, b1_ = c * B_S, (c + 1) * B_S
    s = np.zeros((16, B_S), np.float32)
    s[0:10] = input_c[b0:b1_].T
    s[10:14] = input_d[b0:b1_].T
    s[14] = 1.0
    nt = np.ascontiguousarray(noise[b0:b1_].T)
    return {"S": s.astype(BF16), "NT": nt.astype(BF16)}


_NC_LOCK = threading.Lock()
_NC_CACHE = {}


def _get_nc():
    with _NC_LOCK:
        if "nc" not in _NC_CACHE:
            _NC_CACHE["nc"] = build_nc()
        return _NC_CACHE["nc"]


def kernel(noise, input_c, input_d, noise_d, mu, sigma, Wc, W1, b1, W2, b2):
    noise = np.asarray(noise, np.float32)
    input_c = np.asarray(input_c, np.float32)
    input_d = np.asarray(input_d, np.float32)
    b2 = np.asarray(b2, np.float32)
    w = prep_weights(
        np.asarray(noise_d, np.float32),
        np.asarray(mu, np.float32),
        np.asarray(sigma, np.float32),
        np.asarray(Wc, np.float32),
        np.asarray(W1, np.float32),
        np.asarray(b1, np.float32),
        np.asarray(W2, np.float32),
        b2,
    )
    in_maps = []
    for c in range(N_CORES):
        m = prep_core_inputs(noise, input_c, input_d, c)
        m.update(w)
        in_maps.append(m)

    nc = _get_nc()
    res = run_bass_kernel_spmd(nc, in_maps, list(range(N_CORES)))
    out_p = np.concatenate(
        [res.results[c]["OUT"].T for c in range(N_CORES)], axis=0
    )
    # undo the device row permutation; b2 is added on host (device
    # computes s_i = h_i @ W2_i only)
    perm = COLLECT_A + COLLECT_B
    out = np.empty_like(out_p)
    out[:, perm] = out_p
    out = out + b2[None, :]
    return np.ascontiguousarray(out, np.float32)
